# revision 13
# baseline (speedup 1.0000x reference)
"""AttentiveFP hetero-molecular GNN on 8 TRN2 NeuronCores.

Structure exploited: edge_src == arange(N) (one edge per atom) and edge_dst is a
sorted graph id in [0,256) -> the GATConv is per-graph softmax pooling of atoms.
Data-parallel over graphs: 32 graphs/core. All segment ops become matmuls against
a per-core atom->graph one-hot (index-derived, built host-side). xs = x @ Wsrc is
timestep-invariant: computed once on device, stored bf16 (la resident in SBUF,
pa streamed via DRAM). Layouts are pre-transposed host-side so no on-device
transposes are needed; GRU/readout run in [channel, graph] layout.
"""
import sys
sys.path.insert(0, '/opt/trn_rl_repo')

import numpy as np

import concourse.bass as bass
import concourse.bacc as bacc
import concourse.mybir as mybir
from concourse import tile
from concourse.bass_utils import run_bass_kernel_spmd

H, C, T, B = 4, 128, 3, 256
LAST_EXEC_NS = None
NCORES = 8
GPC = B // NCORES          # 32 graphs per core
F32 = mybir.dt.float32
BF16 = mybir.dt.bfloat16
AF = mybir.ActivationFunctionType
ALU = mybir.AluOpType


def _branch(nc, tc, pools, br, Tt, resident):
    """Emit one molecular branch. br: 'pa'|'la'. Tt: atom tiles per core."""
    const, work, pmm, pacc, psm = pools
    Np = Tt * 128
    TO = (Tt + 3) // 4  # packed OHT column tiles

    # ---- DRAM params (per-core shards, same shapes on all cores) ----
    xT = nc.declare_dram_parameter(f"xT_{br}", [128, Np], F32, isOutput=False)
    OH_d = nc.declare_dram_parameter(f"OH_{br}", [128, Tt * GPC], F32, isOutput=False)
    OHT_d = nc.declare_dram_parameter(f"OHT_{br}", [GPC, Tt * 128], F32, isOutput=False)
    embT_d = nc.declare_dram_parameter(f"embT_{br}", [128, GPC], F32, isOutput=False)
    Wsrc_d = nc.declare_dram_parameter(f"Wsrc_{br}", [128, H * C], F32, isOutput=False)
    Wdst_d = nc.declare_dram_parameter(f"Wdst_{br}", [128, H * C], F32, isOutput=False)
    atts_d = nc.declare_dram_parameter(f"atts_{br}", [128, H * C], F32, isOutput=False)
    attd_d = nc.declare_dram_parameter(f"attd_{br}", [128, H * C], F32, isOutput=False)
    bias_d = nc.declare_dram_parameter(f"bias_{br}", [128, 1], F32, isOutput=False)
    WihT_d = nc.declare_dram_parameter(f"WihT_{br}", [128, 3 * C], F32, isOutput=False)
    WhhT_d = nc.declare_dram_parameter(f"WhhT_{br}", [128, 3 * C], F32, isOutput=False)
    bihT_d = nc.declare_dram_parameter(f"bihT_{br}", [128, 3], F32, isOutput=False)
    bhhT_d = nc.declare_dram_parameter(f"bhhT_{br}", [128, 3], F32, isOutput=False)
    linWT_d = nc.declare_dram_parameter(f"linWT_{br}", [128, C], F32, isOutput=False)
    linb_d = nc.declare_dram_parameter(f"linb_{br}", [128, 1], F32, isOutput=False)
    if not resident:
        xs_dram = nc.dram_tensor(f"xs_dram_{br}", [Np, H * C], BF16)

    # ---- resident SBUF constants ----
    Wsrc = const.tile([128, H * C], F32, tag=f"Wsrc{br}")
    Wdst = const.tile([128, H * C], F32, tag=f"Wdst{br}")
    atts = const.tile([128, H * C], F32, tag=f"atts{br}")
    attd = const.tile([128, H * C], F32, tag=f"attd{br}")
    attb = const.tile([128, H * C], F32, tag=f"attb{br}")
    UV = const.tile([128, 2 * H], F32, tag=f"UV{br}")  # U cols 0:4, V cols 4:8
    bias = const.tile([128, 1], F32, tag=f"bias{br}")
    WihT = const.tile([128, 3 * C], F32, tag=f"WihT{br}")
    WhhT = const.tile([128, 3 * C], F32, tag=f"WhhT{br}")
    bihT = const.tile([128, 3], F32, tag=f"bihT{br}")
    bhhT = const.tile([128, 3], F32, tag=f"bhhT{br}")
    bsum = const.tile([128, 3], F32, tag=f"bsum{br}")
    linWT = const.tile([128, C], F32, tag=f"linWT{br}")
    linb = const.tile([128, 1], F32, tag=f"linb{br}")
    OH = const.tile([128, Tt * GPC], F32, tag=f"OH{br}")
    OHT = const.tile([GPC, Tt * 128], F32, tag=f"OHT{br}")
    a_s = const.tile([128, Tt * H], F32, tag=f"as{br}")
    ex_all = const.tile([128, Tt * H], F32, tag=f"ex{br}")
    embT = const.tile([128, GPC], F32, tag=f"embT{br}")
    if resident:
        xs_all = const.tile([128, Tt * H * C], BF16, tag=f"xs{br}")

    nc.gpsimd.dma_start(Wsrc[:], Wsrc_d[:])
    nc.gpsimd.dma_start(Wdst[:], Wdst_d[:])
    nc.gpsimd.dma_start(atts[:], atts_d[:])
    nc.gpsimd.dma_start(attd[:], attd_d[:])
    nc.gpsimd.dma_start(bias[:], bias_d[:])
    nc.gpsimd.dma_start(WihT[:], WihT_d[:])
    nc.gpsimd.dma_start(WhhT[:], WhhT_d[:])
    nc.gpsimd.dma_start(bihT[:], bihT_d[:])
    nc.gpsimd.dma_start(bhhT[:], bhhT_d[:])
    nc.gpsimd.dma_start(linWT[:], linWT_d[:])
    nc.gpsimd.dma_start(linb[:], linb_d[:])
    nc.gpsimd.dma_start(OH[:], OH_d[:])
    nc.gpsimd.dma_start(OHT[:], OHT_d[:])
    nc.gpsimd.dma_start(embT[:], embT_d[:])
    nc.vector.tensor_add(bsum[:], bihT[:], bhhT[:])

    # U[k,h] = sum_c Wsrc[k,hC+c]*atts[h,c]; V likewise from Wdst/attd.
    for (Wt, at, col) in ((Wsrc, atts, 0), (Wdst, attd, H)):
        nc.vector.tensor_mul(attb[:], Wt[:], at[:])
        a3 = attb[:].rearrange("p (h c) -> p h c", h=H)
        nc.vector.tensor_reduce(UV[:, col:col + H], a3,
                                mybir.AxisListType.X, ALU.add)

    # ---- phase A: xs = x @ Wsrc (once), a_s = x @ U ----
    for t in range(Tt):
        xTt = work.tile([128, 128], F32, tag="xTt")
        nc.sync.dma_start(xTt[:], xT[:, t * 128:(t + 1) * 128])
        pxs = pmm.tile([128, H * C], F32, tag="pxs")
        nc.tensor.matmul(pxs[:], xTt[:], Wsrc[:], start=True, stop=True)
        pas = psm.tile([128, H], F32, tag="sm")
        nc.tensor.matmul(pas[:], xTt[:], UV[:, 0:H], start=True, stop=True)
        if resident:
            nc.vector.tensor_copy(xs_all[:, t * H * C:(t + 1) * H * C], pxs[:])
        else:
            xsw = work.tile([128, H * C], BF16, tag="xsw")
            nc.vector.tensor_copy(xsw[:], pxs[:])
            nc.sync.dma_start(xs_dram[t * 128:(t + 1) * 128, :], xsw[:])
        nc.scalar.activation(a_s[:, t * H:(t + 1) * H], pas[:], AF.Copy)

    # ---- phase B: T timesteps ----
    for step in range(T):
        pad = psm.tile([GPC, H], F32, tag="sm")
        nc.tensor.matmul(pad[:], embT[:], UV[:, H:2 * H], start=True, stop=True)
        ad = work.tile([GPC, H], F32, tag="ad")
        nc.scalar.activation(ad[:], pad[:], AF.Copy)

        pden = pacc.tile([GPC, H], F32, tag="acc")
        for t in range(Tt):
            oht = OHT[:, t * 128:(t + 1) * 128]
            padb = psm.tile([128, H], F32, tag="sm")
            nc.tensor.matmul(padb[:], oht, ad[:], start=True, stop=True)
            alf = work.tile([128, H], F32, tag="alf")
            nc.vector.tensor_add(alf[:], a_s[:, t * H:(t + 1) * H], padb[:])
            nc.scalar.activation(alf[:], alf[:], AF.Lrelu, alpha=0.01)
            nc.scalar.activation(ex_all[:, t * H:(t + 1) * H], alf[:], AF.Exp)
            nc.tensor.matmul(pden[:], OH[:, t * GPC:(t + 1) * GPC],
                             ex_all[:, t * H:(t + 1) * H],
                             start=(t == 0), stop=(t == Tt - 1))
        rden = work.tile([GPC, H], F32, tag="rden")
        with nc.allow_low_precision(reason="softmax denom reciprocal, bf16 ok"):
            nc.vector.reciprocal(rden[:], pden[:])

        pout = pacc.tile([128, H * GPC], F32, tag="acc")
        for t in range(Tt):
            oht = OHT[:, t * 128:(t + 1) * 128]
            prdb = psm.tile([128, H], F32, tag="sm")
            nc.tensor.matmul(prdb[:], oht, rden[:], start=True, stop=True)
            wv = work.tile([128, H], F32, tag="wv")
            nc.vector.tensor_mul(wv[:], ex_all[:, t * H:(t + 1) * H], prdb[:])
            ohw = work.tile([128, H * GPC], BF16, tag="ohw")
            for h in range(H):
                eng = nc.vector if h % 2 == 0 else nc.scalar
                if h % 2 == 0:
                    nc.vector.tensor_scalar_mul(
                        ohw[:, h * GPC:(h + 1) * GPC],
                        OH[:, t * GPC:(t + 1) * GPC], wv[:, h:h + 1])
                else:
                    nc.scalar.activation(
                        ohw[:, h * GPC:(h + 1) * GPC],
                        OH[:, t * GPC:(t + 1) * GPC], AF.Copy,
                        scale=wv[:, h:h + 1])
            if resident:
                xs_t = xs_all[:, t * H * C:(t + 1) * H * C]
            else:
                xsr = work.tile([128, H * C], BF16, tag="xsr")
                nc.sync.dma_start(xsr[:], xs_dram[t * 128:(t + 1) * 128, :])
                xs_t = xsr[:]
            for h in range(H):
                nc.tensor.matmul(
                    pout[:, h * GPC:(h + 1) * GPC],
                    xs_t[:, h * C:(h + 1) * C],
                    ohw[:, h * GPC:(h + 1) * GPC],
                    start=(t == 0), stop=(t == Tt - 1))

        # gat.T = mean_h + bias ; hT = elu(gat.T)
        s1 = work.tile([128, GPC], F32, tag="s1")
        nc.scalar.activation(s1[:], pout[:, 0:GPC], AF.Copy)
        nc.vector.tensor_add(s1[:], s1[:], pout[:, GPC:2 * GPC])
        nc.vector.tensor_add(s1[:], s1[:], pout[:, 2 * GPC:3 * GPC])
        nc.vector.tensor_add(s1[:], s1[:], pout[:, 3 * GPC:4 * GPC])
        gat = work.tile([128, GPC], F32, tag="gat")
        nc.vector.tensor_scalar(gat[:], s1[:], 0.25, bias[:, 0:1], ALU.mult, ALU.add)
        e1 = work.tile([128, GPC], F32, tag="e1")
        nc.scalar.activation(e1[:], gat[:], AF.Relu)
        e2 = work.tile([128, GPC], F32, tag="e2")
        nc.scalar.activation(e2[:], gat[:], AF.Exp)
        nc.vector.tensor_scalar(e2[:], e2[:], 1.0, -1.0, ALU.min, ALU.add)
        hT = work.tile([128, GPC], F32, tag="hT")
        nc.vector.tensor_add(hT[:], e1[:], e2[:])

        # GRU in [c, g] layout
        pg = pacc.tile([128, 6 * GPC], F32, tag="acc")
        for j in range(3):
            nc.tensor.matmul(pg[:, j * GPC:(j + 1) * GPC],
                             WihT[:, j * C:(j + 1) * C], hT[:],
                             start=True, stop=True)
            nc.tensor.matmul(pg[:, (3 + j) * GPC:(4 + j) * GPC],
                             WhhT[:, j * C:(j + 1) * C], embT[:],
                             start=True, stop=True)
        ghs = work.tile([128, 3 * GPC], F32, tag="ghs")
        nc.scalar.activation(ghs[:], pg[:, 3 * GPC:6 * GPC], AF.Copy)
        rz = work.tile([128, 2 * GPC], F32, tag="rz")
        for j in range(2):  # r, z
            tsum = work.tile([128, GPC], F32, tag="tsum")
            nc.vector.tensor_add(tsum[:], ghs[:, j * GPC:(j + 1) * GPC],
                                 pg[:, j * GPC:(j + 1) * GPC])
            nc.scalar.activation(rz[:, j * GPC:(j + 1) * GPC], tsum[:],
                                 AF.Sigmoid, bias=bsum[:, j:j + 1])
        hn = work.tile([128, GPC], F32, tag="hn")
        nc.vector.tensor_scalar_add(hn[:], ghs[:, 2 * GPC:3 * GPC], bhhT[:, 2:3])
        nc.vector.tensor_mul(hn[:], rz[:, 0:GPC], hn[:])
        nc.vector.tensor_add(hn[:], pg[:, 2 * GPC:3 * GPC], hn[:])
        nv = work.tile([128, GPC], F32, tag="nv")
        nc.scalar.activation(nv[:], hn[:], AF.Tanh, bias=bihT[:, 2:3])
        d = work.tile([128, GPC], F32, tag="d")
        nc.vector.tensor_sub(d[:], embT[:], nv[:])
        nc.vector.tensor_mul(d[:], rz[:, GPC:2 * GPC], d[:])
        nc.vector.tensor_add(d[:], nv[:], d[:])
        nc.scalar.activation(embT[:], d[:], AF.Relu)

    # readout yT = linW @ embT + linb
    py = psm.tile([128, GPC], F32, tag="sm")
    nc.tensor.matmul(py[:], linWT[:], embT[:], start=True, stop=True)
    yT = work.tile([128, GPC], F32, tag=f"yT{br}")
    nc.vector.tensor_scalar_add(yT[:], py[:], linb[:, 0:1])
    return yT


def build_nc(Tt_pa, Tt_la):
    nc = bacc.Bacc(None, target_bir_lowering=False)
    out_d = nc.declare_dram_parameter("out", [128, 2 * GPC], F32, isOutput=True)
    with tile.TileContext(nc) as tc:
        with (
            tc.tile_pool(name="const", bufs=1) as const,
            tc.tile_pool(name="work", bufs=3) as work,
            tc.tile_pool(name="pmm", bufs=2, space=bass.MemorySpace.PSUM) as pmm,
            tc.tile_pool(name="pacc", bufs=2, space=bass.MemorySpace.PSUM) as pacc,
            tc.tile_pool(name="psm", bufs=4, space=bass.MemorySpace.PSUM) as psm,
        ):
            pools = (const, work, pmm, pacc, psm)
            yT_pa = _branch(nc, tc, pools, "pa", Tt_pa, resident=False)
            yT_la = _branch(nc, tc, pools, "la", Tt_la, resident=True)
            nc.sync.dma_start(out_d[:, 0:GPC], yT_pa[:])
            nc.sync.dma_start(out_d[:, GPC:2 * GPC], yT_la[:])
    nc.compile()
    return nc


def _host_prep(x, emb, dst, prm):
    """Return per-core in_map fragments for one branch + Tt."""
    N = x.shape[0]
    counts = np.bincount(dst, minlength=B)
    offs = np.concatenate([[0], np.cumsum(counts)]).astype(np.int64)
    core_n = [int(offs[(k + 1) * GPC] - offs[k * GPC]) for k in range(NCORES)]
    Tt = max((n + 127) // 128 for n in core_n)
    Np = Tt * 128
    TO = (Tt + 3) // 4
    frags = []
    for k in range(NCORES):
        a0, a1 = int(offs[k * GPC]), int(offs[(k + 1) * GPC])
        n = a1 - a0
        xp = np.zeros((Np, C), np.float32)
        xp[:n] = x[a0:a1]
        OHf = np.zeros((Np, GPC), np.float32)
        OHf[np.arange(n), dst[a0:a1] - k * GPC] = 1.0
        OH3 = OHf.reshape(Tt, 128, GPC)
        OHcat = np.concatenate([OH3[t] for t in range(Tt)], axis=1)  # [128, Tt*GPC]
        OHT = np.concatenate([OH3[t].T for t in range(Tt)], axis=1)
        frags.append(dict(
            xT=np.ascontiguousarray(xp.T),
            OH=OHcat.astype(np.float32), OHT=OHT,
            embT=np.ascontiguousarray(emb[k * GPC:(k + 1) * GPC].T),
        ))
    shared = dict(
        Wsrc=prm['Wsrc'], Wdst=prm['Wdst'],
        atts=np.repeat(prm['atts'].reshape(1, H * C), 128, axis=0),
        attd=np.repeat(prm['attd'].reshape(1, H * C), 128, axis=0),
        bias=prm['bias'].reshape(C, 1),
        WihT=np.ascontiguousarray(prm['Wih'].T), WhhT=np.ascontiguousarray(prm['Whh'].T),
        bihT=np.ascontiguousarray(prm['bih'].reshape(3, C).T),
        bhhT=np.ascontiguousarray(prm['bhh'].reshape(3, C).T),
        linWT=np.ascontiguousarray(prm['linW'].T),
        linb=prm['linb'].reshape(C, 1),
    )
    return frags, shared, Tt


def _to_bf16(a):
    import jax.numpy as jnp  # only for rounding; ml_dtypes fallback below
    try:
        import ml_dtypes
        return a.astype(ml_dtypes.bfloat16)
    except ImportError:
        return np.asarray(jnp.asarray(a, jnp.bfloat16))


def kernel(**inputs):
    inputs = {k: np.asarray(v) for k, v in inputs.items()}
    maps = [dict() for _ in range(NCORES)]
    Tts = {}
    for br in ('pa', 'la'):
        prm = {k: inputs[f'{k}_{br}'] for k in
               ('Wsrc', 'Wdst', 'atts', 'attd', 'bias', 'Wih', 'Whh',
                'bih', 'bhh', 'linW', 'linb')}
        frags, shared, Tt = _host_prep(
            inputs[f'x_{br}'], inputs[f'emb_{br}'],
            inputs[f'edge_dst_{br}'].astype(np.int64), prm)
        Tts[br] = Tt
        for k in range(NCORES):
            for nm, v in frags[k].items():
                maps[k][f'{nm}_{br}'] = v
            for nm, v in shared.items():
                maps[k][f'{nm}_{br}'] = np.asarray(v, np.float32)
    nc = build_nc(Tts['pa'], Tts['la'])
    rk = run_bass_kernel_spmd(nc, maps, list(range(NCORES)))
    global LAST_EXEC_NS
    LAST_EXEC_NS = rk.exec_time_ns
    res = rk.results
    y_pa = np.zeros((B, C), np.float32)
    y_la = np.zeros((B, C), np.float32)
    for k in range(NCORES):
        o = np.asarray(res[k]['out'])
        y_pa[k * GPC:(k + 1) * GPC] = o[:, :GPC].T
        y_la[k * GPC:(k + 1) * GPC] = o[:, GPC:].T
    return (y_pa, y_la)


# revision 15
# speedup vs baseline: 1.0623x; 1.0623x over previous
"""AttentiveFP hetero-molecular GNN on 8 TRN2 NeuronCores.

Structure exploited: edge_src == arange(N) (one edge per atom) and edge_dst is a
sorted graph id in [0,256) -> the GATConv is per-graph softmax pooling of atoms.
Data-parallel over graphs: 32 graphs/core. All segment ops become matmuls against
a per-core atom->graph one-hot (index-derived, built host-side). xs = x @ Wsrc is
timestep-invariant: computed once on device, stored bf16 (la resident in SBUF,
pa streamed via DRAM). Layouts are pre-transposed host-side so no on-device
transposes are needed; GRU/readout run in [channel, graph] layout.
"""
import sys
sys.path.insert(0, '/opt/trn_rl_repo')

import numpy as np

import concourse.bass as bass
import concourse.bacc as bacc
import concourse.mybir as mybir
from concourse import tile
from concourse.bass_utils import run_bass_kernel_spmd

H, C, T, B = 4, 128, 3, 256
LAST_EXEC_NS = None
NCORES = 8
GPC = B // NCORES          # 32 graphs per core
F32 = mybir.dt.float32
BF16 = mybir.dt.bfloat16
AF = mybir.ActivationFunctionType
ALU = mybir.AluOpType


def _branch(nc, tc, pools, br, Tt, resident):
    """Emit one molecular branch. br: 'pa'|'la'. Tt: atom tiles per core."""
    const, work, pmm, pacc, psm = pools
    Np = Tt * 128
    TO = (Tt + 3) // 4  # packed OHT column tiles

    # ---- DRAM params (per-core shards, same shapes on all cores) ----
    xT = nc.declare_dram_parameter(f"xT_{br}", [128, Np], F32, isOutput=False)
    OH_d = nc.declare_dram_parameter(f"OH_{br}", [128, Tt * GPC], F32, isOutput=False)
    OHT_d = nc.declare_dram_parameter(f"OHT_{br}", [GPC, Tt * 128], F32, isOutput=False)
    embT_d = nc.declare_dram_parameter(f"embT_{br}", [128, GPC], F32, isOutput=False)
    Wsrc_d = nc.declare_dram_parameter(f"Wsrc_{br}", [128, H * C], F32, isOutput=False)
    Wdst_d = nc.declare_dram_parameter(f"Wdst_{br}", [128, H * C], F32, isOutput=False)
    atts_d = nc.declare_dram_parameter(f"atts_{br}", [128, H * C], F32, isOutput=False)
    attd_d = nc.declare_dram_parameter(f"attd_{br}", [128, H * C], F32, isOutput=False)
    bias_d = nc.declare_dram_parameter(f"bias_{br}", [128, 1], F32, isOutput=False)
    WihT_d = nc.declare_dram_parameter(f"WihT_{br}", [128, 3 * C], F32, isOutput=False)
    WhhT_d = nc.declare_dram_parameter(f"WhhT_{br}", [128, 3 * C], F32, isOutput=False)
    bihT_d = nc.declare_dram_parameter(f"bihT_{br}", [128, 3], F32, isOutput=False)
    bhhT_d = nc.declare_dram_parameter(f"bhhT_{br}", [128, 3], F32, isOutput=False)
    linWT_d = nc.declare_dram_parameter(f"linWT_{br}", [128, C], F32, isOutput=False)
    linb_d = nc.declare_dram_parameter(f"linb_{br}", [128, 1], F32, isOutput=False)
    if not resident:
        xs_dram = nc.dram_tensor(f"xs_dram_{br}", [Np, H * C], BF16)

    # ---- resident SBUF constants ----
    Wsrc = const.tile([128, H * C], F32, tag=f"Wsrc{br}")
    Wdst = const.tile([128, H * C], F32, tag=f"Wdst{br}")
    atts = const.tile([128, H * C], F32, tag=f"atts{br}")
    attd = const.tile([128, H * C], F32, tag=f"attd{br}")
    attb = const.tile([128, H * C], F32, tag=f"attb{br}")
    UV = const.tile([128, 2 * H], F32, tag=f"UV{br}")  # U cols 0:4, V cols 4:8
    bias = const.tile([128, 1], F32, tag=f"bias{br}")
    WihT = const.tile([128, 3 * C], F32, tag=f"WihT{br}")
    WhhT = const.tile([128, 3 * C], F32, tag=f"WhhT{br}")
    bihT = const.tile([128, 3], F32, tag=f"bihT{br}")
    bhhT = const.tile([128, 3], F32, tag=f"bhhT{br}")
    bsum = const.tile([128, 3], F32, tag=f"bsum{br}")
    linWT = const.tile([128, C], F32, tag=f"linWT{br}")
    linb = const.tile([128, 1], F32, tag=f"linb{br}")
    OH = const.tile([128, Tt * GPC], F32, tag=f"OH{br}")
    OHT = const.tile([GPC, Tt * 128], F32, tag=f"OHT{br}")
    a_s = const.tile([128, Tt * H], F32, tag=f"as{br}")
    ex_all = const.tile([128, Tt * H], F32, tag=f"ex{br}")
    embT = const.tile([128, GPC], F32, tag=f"embT{br}")
    if resident:
        xs_all = const.tile([128, Tt * H * C], BF16, tag=f"xs{br}")

    nc.gpsimd.dma_start(Wsrc[:], Wsrc_d[:])
    nc.gpsimd.dma_start(Wdst[:], Wdst_d[:])
    nc.gpsimd.dma_start(atts[:], atts_d[:])
    nc.gpsimd.dma_start(attd[:], attd_d[:])
    nc.gpsimd.dma_start(bias[:], bias_d[:])
    nc.gpsimd.dma_start(WihT[:], WihT_d[:])
    nc.gpsimd.dma_start(WhhT[:], WhhT_d[:])
    nc.gpsimd.dma_start(bihT[:], bihT_d[:])
    nc.gpsimd.dma_start(bhhT[:], bhhT_d[:])
    nc.gpsimd.dma_start(linWT[:], linWT_d[:])
    nc.gpsimd.dma_start(linb[:], linb_d[:])
    nc.gpsimd.dma_start(OH[:], OH_d[:])
    nc.gpsimd.dma_start(OHT[:], OHT_d[:])
    nc.gpsimd.dma_start(embT[:], embT_d[:])
    nc.vector.tensor_add(bsum[:], bihT[:], bhhT[:])

    # U[k,h] = sum_c Wsrc[k,hC+c]*atts[h,c]; V likewise from Wdst/attd.
    for (Wt, at, col) in ((Wsrc, atts, 0), (Wdst, attd, H)):
        nc.vector.tensor_mul(attb[:], Wt[:], at[:])
        a3 = attb[:].rearrange("p (h c) -> p h c", h=H)
        nc.vector.tensor_reduce(UV[:, col:col + H], a3,
                                mybir.AxisListType.X, ALU.add)

    # ---- phase A: xs = x @ Wsrc (once), a_s = x @ U ----
    for t in range(Tt):
        xTt = work.tile([128, 128], F32, tag="xTt")
        nc.sync.dma_start(xTt[:], xT[:, t * 128:(t + 1) * 128])
        pxs = pmm.tile([128, H * C], F32, tag="pxs")
        nc.tensor.matmul(pxs[:], xTt[:], Wsrc[:], start=True, stop=True)
        pas = psm.tile([128, H], F32, tag="sm")
        nc.tensor.matmul(pas[:], xTt[:], UV[:, 0:H], start=True, stop=True)
        if resident:
            nc.vector.tensor_copy(xs_all[:, t * H * C:(t + 1) * H * C], pxs[:])
        else:
            xsw = work.tile([128, H * C], BF16, tag="xsw")
            nc.vector.tensor_copy(xsw[:], pxs[:])
            nc.sync.dma_start(xs_dram[t * 128:(t + 1) * 128, :], xsw[:])
        nc.scalar.activation(a_s[:, t * H:(t + 1) * H], pas[:], AF.Copy)

    # ---- phase B: T timesteps ----
    for step in range(T):
        pad = psm.tile([GPC, H], F32, tag="sm")
        nc.tensor.matmul(pad[:], embT[:], UV[:, H:2 * H], start=True, stop=True)
        ad = work.tile([GPC, H], F32, tag="ad")
        nc.scalar.activation(ad[:], pad[:], AF.Copy)

        pden = pacc.tile([GPC, H], F32, tag="acc")
        for t0 in range(0, Tt, 4):
            nt = min(4, Tt - t0)
            padb = psm.tile([128, 4 * H], F32, tag="sm")
            for j in range(nt):
                t = t0 + j
                nc.tensor.matmul(padb[:, j * H:(j + 1) * H],
                                 OHT[:, t * 128:(t + 1) * 128], ad[:],
                                 start=True, stop=True)
            alf = work.tile([128, 4 * H], F32, tag="alf")
            nc.vector.tensor_add(alf[:, :nt * H],
                                 a_s[:, t0 * H:(t0 + nt) * H], padb[:, :nt * H])
            nc.scalar.activation(alf[:, :nt * H], alf[:, :nt * H],
                                 AF.Lrelu, alpha=0.01)
            nc.scalar.activation(ex_all[:, t0 * H:(t0 + nt) * H],
                                 alf[:, :nt * H], AF.Exp)
            for j in range(nt):
                t = t0 + j
                nc.tensor.matmul(pden[:], OH[:, t * GPC:(t + 1) * GPC],
                                 ex_all[:, t * H:(t + 1) * H],
                                 start=(t == 0), stop=(t == Tt - 1))
        rden = work.tile([GPC, H], F32, tag="rden")
        with nc.allow_low_precision(reason="softmax denom reciprocal, bf16 ok"):
            nc.vector.reciprocal(rden[:], pden[:])

        pout = pacc.tile([128, H * GPC], F32, tag="acc")
        wv4 = None
        for t in range(Tt):
            if t % 4 == 0:
                nt = min(4, Tt - t)
                prdb = psm.tile([128, 4 * H], F32, tag="sm")
                for j in range(nt):
                    nc.tensor.matmul(prdb[:, j * H:(j + 1) * H],
                                     OHT[:, (t + j) * 128:(t + j + 1) * 128],
                                     rden[:], start=True, stop=True)
                wv4 = work.tile([128, 4 * H], F32, tag="wv")
                nc.vector.tensor_mul(wv4[:, :nt * H],
                                     ex_all[:, t * H:(t + nt) * H],
                                     prdb[:, :nt * H])
            wv = wv4[:, (t % 4) * H:(t % 4 + 1) * H]
            ohw = work.tile([128, H * GPC], BF16, tag="ohw")
            for h in range(H):
                eng = nc.vector if h % 2 == 0 else nc.scalar
                if h % 2 == 0:
                    nc.vector.tensor_scalar_mul(
                        ohw[:, h * GPC:(h + 1) * GPC],
                        OH[:, t * GPC:(t + 1) * GPC], wv[:, h:h + 1])
                else:
                    nc.scalar.activation(
                        ohw[:, h * GPC:(h + 1) * GPC],
                        OH[:, t * GPC:(t + 1) * GPC], AF.Copy,
                        scale=wv[:, h:h + 1])
            if resident:
                xs_t = xs_all[:, t * H * C:(t + 1) * H * C]
            else:
                xsr = work.tile([128, H * C], BF16, tag="xsr")
                nc.sync.dma_start(xsr[:], xs_dram[t * 128:(t + 1) * 128, :])
                xs_t = xsr[:]
            for h in range(H):
                nc.tensor.matmul(
                    pout[:, h * GPC:(h + 1) * GPC],
                    xs_t[:, h * C:(h + 1) * C],
                    ohw[:, h * GPC:(h + 1) * GPC],
                    start=(t == 0), stop=(t == Tt - 1))

        # gat.T = mean_h + bias ; hT = elu(gat.T)
        s1 = work.tile([128, GPC], F32, tag="s1")
        nc.scalar.activation(s1[:], pout[:, 0:GPC], AF.Copy)
        nc.vector.tensor_add(s1[:], s1[:], pout[:, GPC:2 * GPC])
        nc.vector.tensor_add(s1[:], s1[:], pout[:, 2 * GPC:3 * GPC])
        nc.vector.tensor_add(s1[:], s1[:], pout[:, 3 * GPC:4 * GPC])
        gat = work.tile([128, GPC], F32, tag="gat")
        nc.vector.tensor_scalar(gat[:], s1[:], 0.25, bias[:, 0:1], ALU.mult, ALU.add)
        e1 = work.tile([128, GPC], F32, tag="e1")
        nc.scalar.activation(e1[:], gat[:], AF.Relu)
        e2 = work.tile([128, GPC], F32, tag="e2")
        nc.scalar.activation(e2[:], gat[:], AF.Exp)
        nc.vector.tensor_scalar(e2[:], e2[:], 1.0, -1.0, ALU.min, ALU.add)
        hT = work.tile([128, GPC], F32, tag="hT")
        nc.vector.tensor_add(hT[:], e1[:], e2[:])

        # GRU in [c, g] layout
        pg = pacc.tile([128, 6 * GPC], F32, tag="acc")
        for j in range(3):
            nc.tensor.matmul(pg[:, j * GPC:(j + 1) * GPC],
                             WihT[:, j * C:(j + 1) * C], hT[:],
                             start=True, stop=True)
            nc.tensor.matmul(pg[:, (3 + j) * GPC:(4 + j) * GPC],
                             WhhT[:, j * C:(j + 1) * C], embT[:],
                             start=True, stop=True)
        ghs = work.tile([128, 3 * GPC], F32, tag="ghs")
        nc.scalar.activation(ghs[:], pg[:, 3 * GPC:6 * GPC], AF.Copy)
        rz = work.tile([128, 2 * GPC], F32, tag="rz")
        for j in range(2):  # r, z
            tsum = work.tile([128, GPC], F32, tag="tsum")
            nc.vector.tensor_add(tsum[:], ghs[:, j * GPC:(j + 1) * GPC],
                                 pg[:, j * GPC:(j + 1) * GPC])
            nc.scalar.activation(rz[:, j * GPC:(j + 1) * GPC], tsum[:],
                                 AF.Sigmoid, bias=bsum[:, j:j + 1])
        hn = work.tile([128, GPC], F32, tag="hn")
        nc.vector.tensor_scalar_add(hn[:], ghs[:, 2 * GPC:3 * GPC], bhhT[:, 2:3])
        nc.vector.tensor_mul(hn[:], rz[:, 0:GPC], hn[:])
        nc.vector.tensor_add(hn[:], pg[:, 2 * GPC:3 * GPC], hn[:])
        nv = work.tile([128, GPC], F32, tag="nv")
        nc.scalar.activation(nv[:], hn[:], AF.Tanh, bias=bihT[:, 2:3])
        d = work.tile([128, GPC], F32, tag="d")
        nc.vector.tensor_sub(d[:], embT[:], nv[:])
        nc.vector.tensor_mul(d[:], rz[:, GPC:2 * GPC], d[:])
        nc.vector.tensor_add(d[:], nv[:], d[:])
        nc.scalar.activation(embT[:], d[:], AF.Relu)

    # readout yT = linW @ embT + linb
    py = psm.tile([128, GPC], F32, tag="sm")
    nc.tensor.matmul(py[:], linWT[:], embT[:], start=True, stop=True)
    yT = work.tile([128, GPC], F32, tag=f"yT{br}")
    nc.vector.tensor_scalar_add(yT[:], py[:], linb[:, 0:1])
    return yT


def build_nc(Tt_pa, Tt_la):
    nc = bacc.Bacc(None, target_bir_lowering=False)
    out_d = nc.declare_dram_parameter("out", [128, 2 * GPC], F32, isOutput=True)
    with tile.TileContext(nc) as tc:
        with (
            tc.tile_pool(name="const", bufs=1) as const,
            tc.tile_pool(name="work", bufs=3) as work,
            tc.tile_pool(name="pmm", bufs=2, space=bass.MemorySpace.PSUM) as pmm,
            tc.tile_pool(name="pacc", bufs=2, space=bass.MemorySpace.PSUM) as pacc,
            tc.tile_pool(name="psm", bufs=4, space=bass.MemorySpace.PSUM) as psm,
        ):
            pools = (const, work, pmm, pacc, psm)
            yT_pa = _branch(nc, tc, pools, "pa", Tt_pa, resident=False)
            yT_la = _branch(nc, tc, pools, "la", Tt_la, resident=True)
            nc.sync.dma_start(out_d[:, 0:GPC], yT_pa[:])
            nc.sync.dma_start(out_d[:, GPC:2 * GPC], yT_la[:])
    nc.compile()
    return nc


def _host_prep(x, emb, dst, prm):
    """Return per-core in_map fragments for one branch + Tt."""
    N = x.shape[0]
    counts = np.bincount(dst, minlength=B)
    offs = np.concatenate([[0], np.cumsum(counts)]).astype(np.int64)
    core_n = [int(offs[(k + 1) * GPC] - offs[k * GPC]) for k in range(NCORES)]
    Tt = max((n + 127) // 128 for n in core_n)
    Np = Tt * 128
    TO = (Tt + 3) // 4
    frags = []
    for k in range(NCORES):
        a0, a1 = int(offs[k * GPC]), int(offs[(k + 1) * GPC])
        n = a1 - a0
        xp = np.zeros((Np, C), np.float32)
        xp[:n] = x[a0:a1]
        OHf = np.zeros((Np, GPC), np.float32)
        OHf[np.arange(n), dst[a0:a1] - k * GPC] = 1.0
        OH3 = OHf.reshape(Tt, 128, GPC)
        OHcat = np.concatenate([OH3[t] for t in range(Tt)], axis=1)  # [128, Tt*GPC]
        OHT = np.concatenate([OH3[t].T for t in range(Tt)], axis=1)
        frags.append(dict(
            xT=np.ascontiguousarray(xp.T),
            OH=OHcat.astype(np.float32), OHT=OHT,
            embT=np.ascontiguousarray(emb[k * GPC:(k + 1) * GPC].T),
        ))
    shared = dict(
        Wsrc=prm['Wsrc'], Wdst=prm['Wdst'],
        atts=np.repeat(prm['atts'].reshape(1, H * C), 128, axis=0),
        attd=np.repeat(prm['attd'].reshape(1, H * C), 128, axis=0),
        bias=prm['bias'].reshape(C, 1),
        WihT=np.ascontiguousarray(prm['Wih'].T), WhhT=np.ascontiguousarray(prm['Whh'].T),
        bihT=np.ascontiguousarray(prm['bih'].reshape(3, C).T),
        bhhT=np.ascontiguousarray(prm['bhh'].reshape(3, C).T),
        linWT=np.ascontiguousarray(prm['linW'].T),
        linb=prm['linb'].reshape(C, 1),
    )
    return frags, shared, Tt


def _to_bf16(a):
    import jax.numpy as jnp  # only for rounding; ml_dtypes fallback below
    try:
        import ml_dtypes
        return a.astype(ml_dtypes.bfloat16)
    except ImportError:
        return np.asarray(jnp.asarray(a, jnp.bfloat16))


def kernel(**inputs):
    inputs = {k: np.asarray(v) for k, v in inputs.items()}
    maps = [dict() for _ in range(NCORES)]
    Tts = {}
    for br in ('pa', 'la'):
        prm = {k: inputs[f'{k}_{br}'] for k in
               ('Wsrc', 'Wdst', 'atts', 'attd', 'bias', 'Wih', 'Whh',
                'bih', 'bhh', 'linW', 'linb')}
        frags, shared, Tt = _host_prep(
            inputs[f'x_{br}'], inputs[f'emb_{br}'],
            inputs[f'edge_dst_{br}'].astype(np.int64), prm)
        Tts[br] = Tt
        for k in range(NCORES):
            for nm, v in frags[k].items():
                maps[k][f'{nm}_{br}'] = v
            for nm, v in shared.items():
                maps[k][f'{nm}_{br}'] = np.asarray(v, np.float32)
    nc = build_nc(Tts['pa'], Tts['la'])
    rk = run_bass_kernel_spmd(nc, maps, list(range(NCORES)))
    global LAST_EXEC_NS
    LAST_EXEC_NS = rk.exec_time_ns
    res = rk.results
    y_pa = np.zeros((B, C), np.float32)
    y_la = np.zeros((B, C), np.float32)
    for k in range(NCORES):
        o = np.asarray(res[k]['out'])
        y_pa[k * GPC:(k + 1) * GPC] = o[:, :GPC].T
        y_la[k * GPC:(k + 1) * GPC] = o[:, GPC:].T
    return (y_pa, y_la)
